# revision 8
# baseline (speedup 1.0000x reference)
"""Trainium2 Bass kernel for nn_DerivedMLP (1,2,64,2,512,512) -> (1,64).

Computation (per the original nn.Module):
  x: (1, 2, 64, 2, 512, 512) f32; channel 0 of dim1 holds the [n, phi] fields.
  gamma[t] = -mean(n[t] * d(phi[t])/dy)        (numpy.gradient semantics on y)
  feats    = stack([input_derived, gamma])     -> (2, 64)
  out      = w2 @ gelu_tanh(w1 @ feats + b1) + b2   (1x1 convs over t)

Sharding: the computation is fully independent per time step t (the MLP is a
1x1 conv over t), so t is sharded across the 8 NeuronCores: core k handles
t in [8k, 8k+8).  Zero communication; each core reads a contiguous 16 MB
slice of x (only channel 0 is ever read — half the nominal input).  The tiny
MLP weights are replicated; each core applies the MLP to its own 8 time
steps and the host concatenates the 8 per-core (1, 8) outputs into (1, 64).

Per-core kernel (Tile framework, memory-bound at ~16 MB HBM reads):
  - 17 HWDGE DMAs: per t, phi before n (the stencil needs phi first); p7
    hoisted before n6 and n7 split into 4 chunks so the final multiplies
    start as soon as each chunk lands.
  - DVE: central-difference stencil (512x512 image as one (128, 2048) tile;
    y-segment edges fixed with strided one-sided ops), then prod = n*d
    in place.
  - ACT: free-axis reduction fused into a Copy activation (accum_out);
    the last t's odd chunks reduce on DVE to drain both engines in parallel.
  - PE: ones-matmul for the partition-axis reduction, then the 2x4x1 MLP.
  - gelu matches jax.nn.gelu(approximate=True) exactly via
    0.5*(1+tanh(u)) = sigmoid(2u); gamma's -0.5/dx/N scale is folded into
    w1^T row 0 host-side.
"""

import os
import sys

import numpy as np

for _p in ("/opt/trn_rl_repo",):
    if os.path.isdir(_p) and _p not in sys.path:
        sys.path.insert(0, _p)

# ---- problem constants (hardcoded per contract) ----
DX = 0.1
B, C, T, V, NX, NY = 1, 2, 64, 2, 512, 512
N_CORES = 8
T_PER_CORE = T // N_CORES  # 8
P = 128                    # SBUF partitions
FREE = (NX * NY) // P      # 2048 f32 per partition = whole 512x512 image
SEG = NY                   # 512; partition rows hold 4 y-segments each
LAST = T_PER_CORE - 1
NCH_LAST = 4               # last t processed in 4 column chunks
NCOL = LAST + NCH_LAST     # acc columns
GAMMA_SCALE = -(0.5 / DX) / float(NX * NY)
SQRT_2_OVER_PI = 0.7978845608028654

_CACHE = {}


def _build_nc():
    import concourse.mybir as mybir
    import concourse.tile as tile
    import concourse.bass as bass
    from concourse import bacc

    f32 = mybir.dt.float32
    sub = mybir.AluOpType.subtract
    mult = mybir.AluOpType.mult
    add = mybir.AluOpType.add
    Copy = mybir.ActivationFunctionType.Copy
    Sigmoid = mybir.ActivationFunctionType.Sigmoid

    nc = bacc.Bacc(
        "TRN2", target_bir_lowering=False, debug=False, num_devices=N_CORES
    )

    xs = nc.dram_tensor("xs", (T_PER_CORE, 2, P, FREE), f32, kind="ExternalInput").ap()
    der = nc.dram_tensor("derived", (1, T_PER_CORE), f32, kind="ExternalInput").ap()
    w1t = nc.dram_tensor("w1t", (2, 4), f32, kind="ExternalInput").ap()
    b1 = nc.dram_tensor("b1", (4, 1), f32, kind="ExternalInput").ap()
    w2t = nc.dram_tensor("w2t", (4, 1), f32, kind="ExternalInput").ap()
    b2 = nc.dram_tensor("b2", (1, 1), f32, kind="ExternalInput").ap()
    out = nc.dram_tensor("out", (1, T_PER_CORE), f32, kind="ExternalOutput").ap()

    with tile.TileContext(nc) as tc:
        with (
            tc.tile_pool(name="io", bufs=4) as io,
            tc.tile_pool(name="small", bufs=1) as small,
            tc.tile_pool(name="ps", bufs=1, space=bass.MemorySpace.PSUM) as ps,
        ):
            # tiny replicated MLP weights + derived row — loaded up-front
            w1t_s = small.tile([2, 4], f32)
            b1_s = small.tile([4, 1], f32)
            w2t_s = small.tile([4, 1], f32)
            b2_s = small.tile([1, 1], f32)
            nc.sync.dma_start(w1t_s[:], w1t[:])
            nc.sync.dma_start(b1_s[:], b1[:])
            nc.sync.dma_start(w2t_s[:], w2t[:])
            nc.sync.dma_start(b2_s[:], b2[:])
            feats = small.tile([2, T_PER_CORE], f32)
            nc.sync.dma_start(feats[1:2, :], der[:])

            acc = small.tile([P, NCOL], f32)
            ones = small.tile([P, 1], f32)
            nc.vector.memset(ones[:], 1.0)
            # 1-wide dummy Sigmoid: hoists the ACT function-table load off
            # the kernel tail, overlapping it with the DMA stream
            warm = small.tile([1, 1], f32)
            nc.scalar.activation(warm[:], ones[0:1, :], Sigmoid, bias=0.0, scale=1.0)

            # ---- big loads: per t, phi then n; p7 hoisted before n6 and n7
            # split into NCH_LAST chunks so tail compute starts per-chunk ----
            ptiles, ntiles = {}, {}
            order = []
            for t in range(T_PER_CORE):
                order += [("p", t), ("n", t)]
            order = order[:-4] + [("p", 6), ("p", 7), ("n", 6), ("n", 7)]
            for kind, t in order:
                if kind == "p":
                    ptiles[t] = io.tile([P, FREE], f32, tag="p", name=f"p{t}")
                    nc.sync.dma_start(ptiles[t][:], xs[t, 1])
                else:
                    ntiles[t] = io.tile([P, FREE], f32, tag="n", name=f"n{t}")
                    if t == LAST:
                        W = FREE // NCH_LAST
                        for c in range(NCH_LAST):
                            nc.sync.dma_start(
                                ntiles[t][:, c * W : (c + 1) * W],
                                xs[t, 0][:, c * W : (c + 1) * W],
                            )
                    else:
                        nc.sync.dma_start(ntiles[t][:], xs[t, 0])

            # ---- stencil + product + reduction ----
            col = 0
            for t in range(T_PER_CORE):
                nch = 1 if t < LAST else NCH_LAST
                W = FREE // nch
                ptile, ntile = ptiles[t], ntiles[t]
                for c in range(nch):
                    g0 = c * W
                    dc = io.tile([P, W], f32, tag=f"d{c}_{nch}", name=f"d{t}_{c}")
                    # central diff for chunk-local interior cols (y-segment
                    # edge cols are overwritten below; chunks are whole
                    # segments so no cross-chunk fixups are needed)
                    nc.vector.tensor_tensor(
                        dc[:, 1 : W - 1],
                        ptile[:, g0 + 2 : g0 + W],
                        ptile[:, g0 : g0 + W - 2],
                        sub,
                    )
                    # y-segment left edges: 2*(p[g+1]-p[g])
                    nc.vector.tensor_tensor(
                        dc[:, 0:W:SEG],
                        ptile[:, g0 + 1 : g0 + W : SEG],
                        ptile[:, g0 : g0 + W : SEG],
                        sub,
                    )
                    nc.vector.tensor_scalar_mul(dc[:, 0:W:SEG], dc[:, 0:W:SEG], 2.0)
                    # y-segment right edges: 2*(p[g]-p[g-1])
                    nc.vector.tensor_tensor(
                        dc[:, SEG - 1 : W : SEG],
                        ptile[:, g0 + SEG - 1 : g0 + W : SEG],
                        ptile[:, g0 + SEG - 2 : g0 + W : SEG],
                        sub,
                    )
                    nc.vector.tensor_scalar_mul(
                        dc[:, SEG - 1 : W : SEG], dc[:, SEG - 1 : W : SEG], 2.0
                    )
                    # dc *= n (in place), then reduce along y into acc[:, col]
                    nc.vector.tensor_mul(dc[:], ntile[:, g0 : g0 + W], dc[:])
                    if t == LAST and c % 2 == 1:
                        # drain ACT and DVE in parallel on the tail
                        nc.vector.reduce_sum(
                            acc[:, col : col + 1], dc[:], axis=mybir.AxisListType.X
                        )
                    else:
                        nc.scalar.activation(
                            dc[:], dc[:], Copy, bias=0.0, scale=1.0,
                            accum_out=acc[:, col : col + 1],
                        )
                    col += 1

            # ---- partition reduction + MLP ----
            gsum = ps.tile([1, NCOL], f32)
            nc.tensor.matmul(gsum[:], ones[:], acc[:], start=True, stop=True)

            # feats row 0 = raw sum per t (gamma scale folded into w1t row 0)
            nc.vector.tensor_copy(feats[0:1, 0:LAST], gsum[:, 0:LAST])
            nc.vector.reduce_sum(
                feats[0:1, LAST : LAST + 1], gsum[:, LAST:NCOL],
                axis=mybir.AxisListType.X,
            )

            # z = w1 @ feats + b1  (bias-add on DVE straight from PSUM)
            hp = ps.tile([4, T_PER_CORE], f32)
            nc.tensor.matmul(hp[:], w1t_s[:], feats[:], start=True, stop=True)
            z = small.tile([4, T_PER_CORE], f32)
            nc.vector.tensor_scalar_add(z[:], hp[:], b1_s[:])
            # gelu_tanh(z) = z * sigmoid(2*sqrt(2/pi) * z*(1 + 0.044715 z^2))
            z2 = small.tile([4, T_PER_CORE], f32)
            nc.vector.tensor_mul(z2[:], z[:], z[:])
            t1 = small.tile([4, T_PER_CORE], f32)
            nc.vector.tensor_scalar(t1[:], z2[:], 0.044715, 1.0, mult, add)
            inner = small.tile([4, T_PER_CORE], f32)
            nc.vector.tensor_mul(inner[:], t1[:], z[:])
            sg = small.tile([4, T_PER_CORE], f32)
            nc.scalar.activation(
                sg[:], inner[:], Sigmoid, bias=0.0, scale=2.0 * SQRT_2_OVER_PI
            )
            h = small.tile([4, T_PER_CORE], f32)
            nc.vector.tensor_mul(h[:], sg[:], z[:])

            # out = w2 @ h + b2
            op_ps = ps.tile([1, T_PER_CORE], f32)
            nc.tensor.matmul(op_ps[:], w2t_s[:], h[:], start=True, stop=True)
            res = small.tile([1, T_PER_CORE], f32)
            nc.vector.tensor_scalar_add(res[:], op_ps[:], b2_s[:])
            nc.sync.dma_start(out[:], res[:])

    nc.compile()
    return nc


def get_nc():
    if "nc" not in _CACHE:
        _CACHE["nc"] = _build_nc()
    return _CACHE["nc"]


def make_in_maps(x, input_derived, w1, b1, w2, b2):
    x = np.asarray(x, dtype=np.float32)
    input_derived = np.asarray(input_derived, dtype=np.float32)
    # w1t: (2, 4) = w1.T with rows swapped to the kernel's (gamma, derived)
    # feature order, gamma row pre-scaled by GAMMA_SCALE (kernel feeds raw
    # stencil sums)
    w1t = np.ascontiguousarray(np.asarray(w1, np.float32).T[::-1]).copy()
    w1t[0, :] *= np.float32(GAMMA_SCALE)
    b1c = np.ascontiguousarray(np.asarray(b1, np.float32).reshape(4, 1))
    w2t = np.ascontiguousarray(np.asarray(w2, np.float32).T)
    b2c = np.ascontiguousarray(np.asarray(b2, np.float32).reshape(1, 1))

    x0 = x[0, 0]  # (64, 2, 512, 512): [t, v, nx, ny]
    in_maps = []
    for k in range(N_CORES):
        t0 = k * T_PER_CORE
        xs_k = np.ascontiguousarray(x0[t0 : t0 + T_PER_CORE]).reshape(
            T_PER_CORE, 2, P, FREE
        )
        der_k = np.ascontiguousarray(input_derived[:, t0 : t0 + T_PER_CORE])
        in_maps.append(
            {"xs": xs_k, "derived": der_k, "w1t": w1t, "b1": b1c, "w2t": w2t, "b2": b2c}
        )
    return in_maps


def kernel(x, input_derived, w1, b1, w2, b2, trace=False):
    from concourse.bass_utils import run_bass_kernel_spmd

    nc = get_nc()
    in_maps = make_in_maps(x, input_derived, w1, b1, w2, b2)
    results = run_bass_kernel_spmd(
        nc, in_maps, core_ids=list(range(N_CORES)), trace=trace
    )
    _CACHE["last_results"] = results
    return np.concatenate([r["out"] for r in results.results], axis=1)


# revision 9
# speedup vs baseline: 1.0922x; 1.0922x over previous
"""Trainium2 Bass kernel for nn_DerivedMLP (1,2,64,2,512,512) -> (1,64).

Computation (per the original nn.Module):
  x: (1, 2, 64, 2, 512, 512) f32; channel 0 of dim1 holds the [n, phi] fields.
  gamma[t] = -mean(n[t] * d(phi[t])/dy)        (numpy.gradient semantics on y)
  feats    = stack([input_derived, gamma])     -> (2, 64)
  out      = w2 @ gelu_tanh(w1 @ feats + b1) + b2   (1x1 convs over t)

Sharding: the computation is fully independent per time step t (the MLP is a
1x1 conv over t), so t is sharded across the 8 NeuronCores: core k handles
t in [8k, 8k+8).  Zero communication; each core reads a contiguous 16 MB
slice of x (only channel 0 is ever read — half the nominal input).  The tiny
MLP weights are replicated; each core applies the MLP to its own 8 time
steps and the host concatenates the 8 per-core (1, 8) outputs into (1, 64).

Per-core kernel (Tile framework, memory-bound at ~16 MB HBM reads):
  - 17 HWDGE DMAs: per t, phi before n (the stencil needs phi first); p7
    hoisted before n6 and n7 split into 4 chunks so the final multiplies
    start as soon as each chunk lands.
  - DVE: central-difference stencil (512x512 image as one (128, 2048) tile;
    y-segment edges fixed with strided one-sided ops), then prod = n*d
    in place.
  - ACT: free-axis reduction fused into a Copy activation (accum_out);
    the last t's odd chunks reduce on DVE to drain both engines in parallel.
  - PE: ones-matmul for the partition-axis reduction, then the 2x4x1 MLP.
  - gelu matches jax.nn.gelu(approximate=True) exactly via
    0.5*(1+tanh(u)) = sigmoid(2u); gamma's -0.5/dx/N scale is folded into
    w1^T row 0 host-side.
"""

import os
import sys

import numpy as np

for _p in ("/opt/trn_rl_repo",):
    if os.path.isdir(_p) and _p not in sys.path:
        sys.path.insert(0, _p)

# Defensive: the bass execution path runs through the axon PJRT plugin; if the
# caller's env pinned JAX_PLATFORMS without axon (and jax isn't initialized
# yet), restore it so jax.devices() can see the NeuronCores.
if (
    os.environ.get("AXON_H4_ENABLED") == "1"
    or os.environ.get("AXON_TERMINAL_JOB_NAME")
) and "jax" not in sys.modules:
    _plat = os.environ.get("JAX_PLATFORMS", "")
    if _plat and "axon" not in _plat:
        os.environ["JAX_PLATFORMS"] = "axon," + _plat

# ---- problem constants (hardcoded per contract) ----
DX = 0.1
B, C, T, V, NX, NY = 1, 2, 64, 2, 512, 512
N_CORES = 8
T_PER_CORE = T // N_CORES  # 8
P = 128                    # SBUF partitions
FREE = (NX * NY) // P      # 2048 f32 per partition = whole 512x512 image
SEG = NY                   # 512; partition rows hold 4 y-segments each
LAST = T_PER_CORE - 1
NCH_LAST = 4               # last t processed in 4 column chunks
NCOL = LAST + NCH_LAST     # acc columns
GAMMA_SCALE = -(0.5 / DX) / float(NX * NY)
SQRT_2_OVER_PI = 0.7978845608028654

_CACHE = {}


def _build_nc():
    import concourse.mybir as mybir
    import concourse.tile as tile
    import concourse.bass as bass
    from concourse import bacc

    f32 = mybir.dt.float32
    sub = mybir.AluOpType.subtract
    mult = mybir.AluOpType.mult
    add = mybir.AluOpType.add
    Copy = mybir.ActivationFunctionType.Copy
    Sigmoid = mybir.ActivationFunctionType.Sigmoid

    nc = bacc.Bacc(
        "TRN2", target_bir_lowering=False, debug=False, num_devices=N_CORES
    )

    xs = nc.dram_tensor("xs", (T_PER_CORE, 2, P, FREE), f32, kind="ExternalInput").ap()
    der = nc.dram_tensor("derived", (1, T_PER_CORE), f32, kind="ExternalInput").ap()
    w1t = nc.dram_tensor("w1t", (2, 4), f32, kind="ExternalInput").ap()
    b1 = nc.dram_tensor("b1", (4, 1), f32, kind="ExternalInput").ap()
    w2t = nc.dram_tensor("w2t", (4, 1), f32, kind="ExternalInput").ap()
    b2 = nc.dram_tensor("b2", (1, 1), f32, kind="ExternalInput").ap()
    out = nc.dram_tensor("out", (1, T_PER_CORE), f32, kind="ExternalOutput").ap()

    with tile.TileContext(nc) as tc:
        with (
            tc.tile_pool(name="io", bufs=4) as io,
            tc.tile_pool(name="small", bufs=1) as small,
            tc.tile_pool(name="ps", bufs=1, space=bass.MemorySpace.PSUM) as ps,
        ):
            # tiny replicated MLP weights + derived row — loaded up-front
            w1t_s = small.tile([2, 4], f32)
            b1_s = small.tile([4, 1], f32)
            w2t_s = small.tile([4, 1], f32)
            b2_s = small.tile([1, 1], f32)
            nc.sync.dma_start(w1t_s[:], w1t[:])
            nc.sync.dma_start(b1_s[:], b1[:])
            nc.sync.dma_start(w2t_s[:], w2t[:])
            nc.sync.dma_start(b2_s[:], b2[:])
            feats = small.tile([2, T_PER_CORE], f32)
            nc.sync.dma_start(feats[1:2, :], der[:])

            acc = small.tile([P, NCOL], f32)
            ones = small.tile([P, 1], f32)
            nc.vector.memset(ones[:], 1.0)
            # 1-wide dummy Sigmoid: hoists the ACT function-table load off
            # the kernel tail, overlapping it with the DMA stream
            warm = small.tile([1, 1], f32)
            nc.scalar.activation(warm[:], ones[0:1, :], Sigmoid, bias=0.0, scale=1.0)

            # ---- big loads: per t, phi then n; p7 hoisted before n6 and n7
            # split into NCH_LAST chunks so tail compute starts per-chunk ----
            ptiles, ntiles = {}, {}
            order = []
            for t in range(T_PER_CORE):
                order += [("p", t), ("n", t)]
            order = order[:-4] + [("p", 6), ("p", 7), ("n", 6), ("n", 7)]
            for kind, t in order:
                if kind == "p":
                    ptiles[t] = io.tile([P, FREE], f32, tag="p", name=f"p{t}")
                    nc.sync.dma_start(ptiles[t][:], xs[t, 1])
                else:
                    ntiles[t] = io.tile([P, FREE], f32, tag="n", name=f"n{t}")
                    if t == LAST:
                        W = FREE // NCH_LAST
                        for c in range(NCH_LAST):
                            nc.sync.dma_start(
                                ntiles[t][:, c * W : (c + 1) * W],
                                xs[t, 0][:, c * W : (c + 1) * W],
                            )
                    else:
                        nc.sync.dma_start(ntiles[t][:], xs[t, 0])

            # ---- stencil + product + reduction ----
            col = 0
            for t in range(T_PER_CORE):
                nch = 1 if t < LAST else NCH_LAST
                W = FREE // nch
                ptile, ntile = ptiles[t], ntiles[t]
                for c in range(nch):
                    g0 = c * W
                    dc = io.tile([P, W], f32, tag=f"d{c}_{nch}", name=f"d{t}_{c}")
                    # central diff for chunk-local interior cols (y-segment
                    # edge cols are overwritten below; chunks are whole
                    # segments so no cross-chunk fixups are needed)
                    nc.vector.tensor_tensor(
                        dc[:, 1 : W - 1],
                        ptile[:, g0 + 2 : g0 + W],
                        ptile[:, g0 : g0 + W - 2],
                        sub,
                    )
                    # y-segment left edges: 2*(p[g+1]-p[g])
                    nc.vector.tensor_tensor(
                        dc[:, 0:W:SEG],
                        ptile[:, g0 + 1 : g0 + W : SEG],
                        ptile[:, g0 : g0 + W : SEG],
                        sub,
                    )
                    nc.vector.tensor_scalar_mul(dc[:, 0:W:SEG], dc[:, 0:W:SEG], 2.0)
                    # y-segment right edges: 2*(p[g]-p[g-1])
                    nc.vector.tensor_tensor(
                        dc[:, SEG - 1 : W : SEG],
                        ptile[:, g0 + SEG - 1 : g0 + W : SEG],
                        ptile[:, g0 + SEG - 2 : g0 + W : SEG],
                        sub,
                    )
                    nc.vector.tensor_scalar_mul(
                        dc[:, SEG - 1 : W : SEG], dc[:, SEG - 1 : W : SEG], 2.0
                    )
                    # dc *= n (in place), then reduce along y into acc[:, col]
                    nc.vector.tensor_mul(dc[:], ntile[:, g0 : g0 + W], dc[:])
                    if t == LAST and c % 2 == 1:
                        # drain ACT and DVE in parallel on the tail
                        nc.vector.reduce_sum(
                            acc[:, col : col + 1], dc[:], axis=mybir.AxisListType.X
                        )
                    else:
                        nc.scalar.activation(
                            dc[:], dc[:], Copy, bias=0.0, scale=1.0,
                            accum_out=acc[:, col : col + 1],
                        )
                    col += 1

            # ---- partition reduction + MLP ----
            gsum = ps.tile([1, NCOL], f32)
            nc.tensor.matmul(gsum[:], ones[:], acc[:], start=True, stop=True)

            # feats row 0 = raw sum per t (gamma scale folded into w1t row 0)
            nc.vector.tensor_copy(feats[0:1, 0:LAST], gsum[:, 0:LAST])
            nc.vector.reduce_sum(
                feats[0:1, LAST : LAST + 1], gsum[:, LAST:NCOL],
                axis=mybir.AxisListType.X,
            )

            # z = w1 @ feats + b1  (bias-add on DVE straight from PSUM)
            hp = ps.tile([4, T_PER_CORE], f32)
            nc.tensor.matmul(hp[:], w1t_s[:], feats[:], start=True, stop=True)
            z = small.tile([4, T_PER_CORE], f32)
            nc.vector.tensor_scalar_add(z[:], hp[:], b1_s[:])
            # gelu_tanh(z) = z * sigmoid(2*sqrt(2/pi) * z*(1 + 0.044715 z^2))
            z2 = small.tile([4, T_PER_CORE], f32)
            nc.vector.tensor_mul(z2[:], z[:], z[:])
            t1 = small.tile([4, T_PER_CORE], f32)
            nc.vector.tensor_scalar(t1[:], z2[:], 0.044715, 1.0, mult, add)
            inner = small.tile([4, T_PER_CORE], f32)
            nc.vector.tensor_mul(inner[:], t1[:], z[:])
            sg = small.tile([4, T_PER_CORE], f32)
            nc.scalar.activation(
                sg[:], inner[:], Sigmoid, bias=0.0, scale=2.0 * SQRT_2_OVER_PI
            )
            h = small.tile([4, T_PER_CORE], f32)
            nc.vector.tensor_mul(h[:], sg[:], z[:])

            # out = w2 @ h + b2
            op_ps = ps.tile([1, T_PER_CORE], f32)
            nc.tensor.matmul(op_ps[:], w2t_s[:], h[:], start=True, stop=True)
            res = small.tile([1, T_PER_CORE], f32)
            nc.vector.tensor_scalar_add(res[:], op_ps[:], b2_s[:])
            nc.sync.dma_start(out[:], res[:])

    nc.compile()
    return nc


def get_nc():
    if "nc" not in _CACHE:
        _CACHE["nc"] = _build_nc()
    return _CACHE["nc"]


def make_in_maps(x, input_derived, w1, b1, w2, b2):
    x = np.asarray(x, dtype=np.float32)
    input_derived = np.asarray(input_derived, dtype=np.float32)
    # w1t: (2, 4) = w1.T with rows swapped to the kernel's (gamma, derived)
    # feature order, gamma row pre-scaled by GAMMA_SCALE (kernel feeds raw
    # stencil sums)
    w1t = np.ascontiguousarray(np.asarray(w1, np.float32).T[::-1]).copy()
    w1t[0, :] *= np.float32(GAMMA_SCALE)
    b1c = np.ascontiguousarray(np.asarray(b1, np.float32).reshape(4, 1))
    w2t = np.ascontiguousarray(np.asarray(w2, np.float32).T)
    b2c = np.ascontiguousarray(np.asarray(b2, np.float32).reshape(1, 1))

    x0 = x[0, 0]  # (64, 2, 512, 512): [t, v, nx, ny]
    in_maps = []
    for k in range(N_CORES):
        t0 = k * T_PER_CORE
        xs_k = np.ascontiguousarray(x0[t0 : t0 + T_PER_CORE]).reshape(
            T_PER_CORE, 2, P, FREE
        )
        der_k = np.ascontiguousarray(input_derived[:, t0 : t0 + T_PER_CORE])
        in_maps.append(
            {"xs": xs_k, "derived": der_k, "w1t": w1t, "b1": b1c, "w2t": w2t, "b2": b2c}
        )
    return in_maps


def kernel(x, input_derived, w1, b1, w2, b2, trace=False):
    from concourse.bass_utils import run_bass_kernel_spmd

    nc = get_nc()
    in_maps = make_in_maps(x, input_derived, w1, b1, w2, b2)
    results = run_bass_kernel_spmd(
        nc, in_maps, core_ids=list(range(N_CORES)), trace=trace
    )
    _CACHE["last_results"] = results
    return np.concatenate([r["out"] for r in results.results], axis=1)
